# revision 6
# baseline (speedup 1.0000x reference)
# Multi-head causal attention (B=4, T=2048, D=1024, H=16, dk=64), fp32 in/out.
#
# V2: bf16 datapath, host-side x transpose, fused 2-head exp, post-exp
# triangle-mask multiply (DVE 4x bf16), unified scope so the tile list
# scheduler overlaps projection (PE-bound) with attention (Act-bound).
#
# Sharding: 8 cores = 4 batches x 2 head-groups (8 heads / 512 cols each).
# Host sums the two head-group partials per batch and adds the constant row
# (bv @ wo + bo), exact because softmax rows sum to 1.

import numpy as np

B, T, D, H, DK = 4, 2048, 1024, 16, 64
NCORES = 8
G = 2               # head groups (tensor-parallel over heads)
C = D // G          # 512 columns per core = 8 heads
NH = C // DK        # heads per core = 8
NIB = T // 512      # 4 query blocks of 512
NJC = T // 128      # 16 key chunks of 128
SCALE = 1.0 / 8.0   # 1/sqrt(dk)

MM_MODE = "bf16"


def _enable_ldw_opt():
    # walrus ships with --enable-ldw-opt=false hardcoded; the LDW pass merges
    # and pull-aheads weight loads, which matters for this bf16 kernel (1184
    # explicit Ldweights). Rewrite the flag on the walrus command line.
    import concourse.bass_utils as bu

    if getattr(bu.run_command, "_ldw_patched", False):
        return
    orig = bu.run_command

    def run_command(argv, **kw):
        argv = ["--enable-ldw-opt=true" if a == "--enable-ldw-opt=false" else a
                for a in argv]
        return orig(argv, **kw)

    run_command._ldw_patched = True
    bu.run_command = run_command


def build_nc(mm_mode=MM_MODE, n_reps=1):
    from contextlib import ExitStack

    import concourse.bass as bass
    import concourse.mybir as mybir
    import concourse.tile as tile
    from concourse import bacc


    f32 = mybir.dt.float32
    f32r = mybir.dt.float32r
    bf16 = mybir.dt.bfloat16
    AF = mybir.ActivationFunctionType

    nc = bacc.Bacc("TRN2", target_bir_lowering=False, debug=False,
                   num_devices=NCORES)

    xT_d = nc.dram_tensor("xT", [D, T], bf16, kind="ExternalInput").ap()
    wq_d = nc.dram_tensor("wq", [D, C], bf16, kind="ExternalInput").ap()
    wk_d = nc.dram_tensor("wk", [D, C], bf16, kind="ExternalInput").ap()
    wv_d = nc.dram_tensor("wv", [D, C], bf16, kind="ExternalInput").ap()
    wo_d = nc.dram_tensor("wo", [C, D], bf16, kind="ExternalInput").ap()
    bq_d = nc.dram_tensor("bq", [C, 1], f32, kind="ExternalInput").ap()
    bk_d = nc.dram_tensor("bk", [C, 1], f32, kind="ExternalInput").ap()
    tri_d = nc.dram_tensor("tri2", [128, 256], f32, kind="ExternalInput").ap()
    vsel_d = nc.dram_tensor("vsel", [128, NJC, NH, NH], f32,
                            kind="ExternalInput").ap()
    hsel_d = nc.dram_tensor("hsel", [72, NH, DK], bf16, kind="ExternalInput").ap()
    out_d = nc.dram_tensor("out", [T, D], f32, kind="ExternalOutput").ap()

    with tile.TileContext(nc) as tc, ExitStack() as st:
        pers = st.enter_context(tc.tile_pool(name="pers", bufs=1))
        work = st.enter_context(tc.tile_pool(name="work", bufs=1))
        epool = st.enter_context(tc.tile_pool(name="epool", bufs=1))
        ypool = st.enter_context(tc.tile_pool(name="ypool", bufs=1))
        npool = st.enter_context(tc.tile_pool(name="npool", bufs=1))
        opool = st.enter_context(tc.tile_pool(name="opool", bufs=1))
        psS = st.enter_context(tc.tile_pool(name="psS", bufs=1, space="PSUM"))
        psP = st.enter_context(tc.tile_pool(name="psP", bufs=1, space="PSUM"))
        psY = st.enter_context(tc.tile_pool(name="psY", bufs=1, space="PSUM"))

        # ---- persistent tiles ----
        xT = [pers.tile([128, T], bf16, name=f"xT{dc}", tag=f"xT{dc}")
              for dc in range(8)]
        qT = [pers.tile([128, T], bf16, name=f"qT{cc}", tag=f"qT{cc}")
              for cc in range(4)]
        kT = [pers.tile([128, T], bf16, name=f"kT{cc}", tag=f"kT{cc}")
              for cc in range(4)]
        # v in natural layout + one-hot denominator columns (head h's denom
        # lands in psum row 64+h of its AV output)
        v_ext = pers.tile([128, NJC, NH, DK + NH], f32r, name="v_ext", tag="v_ext")
        wq_sb = [pers.tile([128, C], bf16, name=f"wq{dc}", tag=f"wq{dc}")
                 for dc in range(8)]
        wk_sb = [pers.tile([128, C], bf16, name=f"wk{dc}", tag=f"wk{dc}")
                 for dc in range(8)]
        wv_sb = [pers.tile([128, C], bf16, name=f"wv{dc}", tag=f"wv{dc}")
                 for dc in range(8)]
        wo_sb = [pers.tile([128, D], bf16, name=f"wo{cc}", tag=f"wo{cc}")
                 for cc in range(4)]
        bq_sb = pers.tile([128, 4], f32, name="bq_sb", tag="bq_sb")
        bk_sb = pers.tile([128, 4], f32, name="bk_sb", tag="bk_sb")
        tri = pers.tile([128, 256], f32r, name="tri", tag="tri")
        hsel = pers.tile([72, NH, DK], bf16, name="hsel", tag="hsel")

        for dc in range(8):
            nc.sync.dma_start(xT[dc][:], xT_d[dc * 128:(dc + 1) * 128, :])
            nc.sync.dma_start(wq_sb[dc][:], wq_d[dc * 128:(dc + 1) * 128, :])
            nc.sync.dma_start(wk_sb[dc][:], wk_d[dc * 128:(dc + 1) * 128, :])
            nc.sync.dma_start(wv_sb[dc][:], wv_d[dc * 128:(dc + 1) * 128, :])
        for cc in range(4):
            nc.sync.dma_start(wo_sb[cc][:], wo_d[cc * 128:(cc + 1) * 128, :])
            nc.sync.dma_start(bq_sb[:, cc:cc + 1], bq_d[cc * 128:(cc + 1) * 128, :])
            nc.sync.dma_start(bk_sb[:, cc:cc + 1], bk_d[cc * 128:(cc + 1) * 128, :])
        nc.sync.dma_start(tri[:], tri_d.bitcast(f32r)[:, :])
        nc.sync.dma_start(hsel[:], hsel_d[:, :, :])
        nc.sync.dma_start(v_ext[:, :, :, DK:DK + NH], vsel_d.bitcast(f32r)[:, :, :, :])

        for rep_ in range(n_reps):
            for ib in range(NIB):
                # ---------------- A(ib): projections ----------------
                for (wsb, dstT, bias_sb) in ((wq_sb, qT, bq_sb), (wk_sb, kT, bk_sb)):
                    for cc in range(4):
                        ps = psP.tile([128, 512], f32, name=f"r{rep_}_psq_{ib}_{cc}",
                                      tag="ps", bufs=2)
                        for dc in range(8):
                            nc.tensor.matmul(
                                ps[:],
                                wsb[dc][:, cc * 128:(cc + 1) * 128],
                                xT[dc][:, ib * 512:(ib + 1) * 512],
                                start=(dc == 0), stop=(dc == 7))
                        nc.vector.tensor_scalar_add(
                            dstT[cc][:, ib * 512:(ib + 1) * 512], ps[:],
                            bias_sb[:, cc:cc + 1])
                for isub in range(4):
                    jr = ib * 4 + isub
                    ps = psP.tile([128, C], f32, name=f"r{rep_}_psv_{ib}_{isub}",
                                  tag="ps", bufs=2)
                    for dc in range(8):
                        nc.tensor.matmul(
                            ps[:],
                            xT[dc][:, jr * 128:(jr + 1) * 128],
                            wv_sb[dc][:],
                            start=(dc == 0), stop=(dc == 7))
                    nc.vector.tensor_copy(
                        v_ext[:, jr, :, 0:DK],
                        ps[:].rearrange("p (h d) -> p h d", d=DK))

                # ---------------- B(ib): attention + output proj ------------
                njc = 4 * (ib + 1)
                yt = [None] * NH
                denacc = npool.tile([72, 512], f32, name=f"r{rep_}_den_{ib}",
                                    tag="den", bufs=2)
                nc.vector.memset(denacc[DK:72, :], 0.0)
                for hp in range(NH // 2):
                    h0, h1 = 2 * hp, 2 * hp + 1
                    psy0 = psY.tile([72, 512], f32, name=f"r{rep_}_psy_{ib}_{h0}",
                                    tag="y", bufs=2)
                    psy1 = psY.tile([72, 512], f32, name=f"r{rep_}_psy_{ib}_{h1}",
                                    tag="y", bufs=2)
                    for jc in range(njc):
                        o = max(0, jc - 4 * ib)
                        i0 = o * 128
                        nw = 512 - i0
                        pss = psS.tile([128, 1024], f32,
                                       name=f"r{rep_}_pss_{ib}_{hp}_{jc}",
                                       tag="pss", bufs=2)
                        nc.tensor.matmul(
                            pss[:, i0:512],
                            kT[hp][0:64, jc * 128:(jc + 1) * 128],
                            qT[hp][0:64, ib * 512 + i0:(ib + 1) * 512],
                            start=True, stop=True, tile_position=(0, 0))
                        nc.tensor.matmul(
                            pss[:, 512 + i0:1024],
                            kT[hp][64:128, jc * 128:(jc + 1) * 128],
                            qT[hp][64:128, ib * 512 + i0:(ib + 1) * 512],
                            start=True, stop=True, tile_position=(64, 0))
                        et = epool.tile([128, 1024], f32r,
                                        name=f"r{rep_}_et_{ib}_{hp}_{jc}",
                                        tag="et", bufs=4)
                        pss_v = pss[:].rearrange("p (s n) -> p s n", s=2)
                        et_v = et[:].rearrange("p (s n) -> p s n", s=2)
                        nc.scalar.activation(et_v[:, :, i0:512], pss_v[:, :, i0:512],
                                             AF.Exp, scale=SCALE)
                        if jc >= 4 * ib:
                            # zero the masked upper-triangle of the diagonal
                            # 128-block (both heads in one 4x-mode bf16 mul)
                            nc.gpsimd.tensor_mul(
                                et_v[:, :, i0:i0 + 128],
                                et_v[:, :, i0:i0 + 128],
                                tri[:].rearrange("p (s n) -> p s n", s=2))
                        nc.tensor.matmul(
                            psy0[:, i0:512], v_ext[:, jc, h0, :],
                            et[:, i0:512],
                            start=(jc == 0), stop=(jc == njc - 1))
                        nc.tensor.matmul(
                            psy1[:, i0:512], v_ext[:, jc, h1, :],
                            et[:, 512 + i0:1024],
                            start=(jc == 0), stop=(jc == njc - 1))
                    for h, psy in ((h0, psy0), (h1, psy1)):
                        y = ypool.tile([DK, 512], bf16, name=f"r{rep_}_yt_{ib}_{h}",
                                       tag="yt", bufs=9)
                        nc.vector.tensor_copy(y[:], psy[0:DK, :])
                        nc.vector.tensor_add(denacc[DK:72, :], denacc[DK:72, :],
                                             psy[DK:72, :])
                        yt[h] = y
                rec = npool.tile([72, 512], f32, name=f"r{rep_}_rec_{ib}",
                                 tag="rec", bufs=2)
                nc.vector.reciprocal(rec[DK:72, :], denacc[DK:72, :])
                rec_r = npool.tile([72, 512], bf16, name=f"r{rep_}_recr_{ib}",
                                   tag="recr", bufs=2)
                nc.vector.tensor_copy(rec_r[DK:72, :], rec[DK:72, :])
                packed = [opool.tile([128, 512], bf16, name=f"r{rep_}_pk_{ib}_{cc}",
                                     tag=f"pk{cc}", bufs=2) for cc in range(4)]
                for h in range(NH):
                    # broadcast recip row 64+h to 64 partitions via a K=8
                    # one-hot selector matmul (base partition 64 is legal)
                    pb = psY.tile([DK, 512], f32, name=f"r{rep_}_pb_{ib}_{h}",
                                  tag="y", bufs=2)
                    nc.tensor.matmul(pb[:], hsel[DK:72, h, :],
                                     rec_r[DK:72, :], start=True, stop=True)
                    if h % 2 == 0:
                        nc.vector.tensor_mul(packed[h // 2][0:64, :],
                                             yt[h][:], pb[:])
                    else:
                        tmp = npool.tile([DK, 512], bf16,
                                         name=f"r{rep_}_tmp_{ib}_{h}",
                                         tag="tmp", bufs=2)
                        nc.vector.tensor_mul(tmp[:], yt[h][:], pb[:])
                        nc.sync.dma_start(packed[h // 2][64:128, :], tmp[:])
                # out[i, n] = sum_c yT[c, i] * wo[c, n]
                for isub in range(4):
                    r0 = (ib * 4 + isub) * 128
                    osb = opool.tile([128, D], f32, name=f"r{rep_}_osb_{ib}_{isub}",
                                     tag="osb", bufs=2)
                    for nb in range(2):
                        pso = psY.tile([128, 512], f32,
                                       name=f"r{rep_}_pso_{ib}_{isub}_{nb}",
                                       tag="y", bufs=2)
                        for cc in range(4):
                            nc.tensor.matmul(
                                pso[:],
                                packed[cc][:, isub * 128:(isub + 1) * 128],
                                wo_sb[cc][:, nb * 512:(nb + 1) * 512],
                                start=(cc == 0), stop=(cc == 3))
                        nc.scalar.copy(osb[:, nb * 512:(nb + 1) * 512], pso[:])
                    nc.sync.dma_start(out_d[r0:r0 + 128, :], osb[:])

    nc.compile()
    return nc


def make_in_maps(x, wq, bq, wk, bk, wv, bv, wo, bo):
    import concourse.mybir as mybir
    bf16 = mybir.dt.np(mybir.dt.bfloat16)

    jj = np.arange(128)[:, None]
    ii = np.arange(128)[None, :]
    tri = (jj <= ii).astype(np.float32)          # keep j <= i (causal)
    tri2 = np.ascontiguousarray(np.concatenate([tri, tri], axis=1).astype(np.float32))
    eye8 = np.eye(8, dtype=np.float32)
    vsel = np.ascontiguousarray(np.broadcast_to(eye8[None, None], (128, NJC, NH, NH)).astype(np.float32))
    hsel = np.zeros((72, NH, DK), dtype=np.float32)
    hsel[DK:72] = eye8[:, :, None]
    hsel = hsel.astype(bf16)

    x = np.asarray(x, np.float32)
    wq = np.asarray(wq, np.float32)
    wk = np.asarray(wk, np.float32)
    wv = np.asarray(wv, np.float32)
    wo = np.asarray(wo, np.float32)

    in_maps = []
    for c in range(NCORES):
        b, g = c // G, c % G
        cs = slice(g * C, (g + 1) * C)
        in_maps.append({
            "xT": np.ascontiguousarray(x[b].T).astype(bf16),
            "wq": np.ascontiguousarray(wq[:, cs]).astype(bf16),
            "wk": np.ascontiguousarray(wk[:, cs]).astype(bf16),
            "wv": np.ascontiguousarray(wv[:, cs]).astype(bf16),
            "wo": np.ascontiguousarray(wo[cs, :]).astype(bf16),
            "bq": np.ascontiguousarray(np.asarray(bq, np.float32)[cs].reshape(C, 1)),
            "bk": np.ascontiguousarray(np.asarray(bk, np.float32)[cs].reshape(C, 1)),
            "tri2": tri2,
            "vsel": vsel,
            "hsel": hsel,
        })
    return in_maps


_NC_CACHE = {}


def _get_nc(mm_mode=MM_MODE):
    if mm_mode not in _NC_CACHE:
        _NC_CACHE[mm_mode] = build_nc(mm_mode)
    return _NC_CACHE[mm_mode]


def kernel(x, mask, wq, bq, wk, bk, wv, bv, wo, bo, _trace=False, _results=None):
    from concourse.bass_utils import run_bass_kernel_spmd

    x = np.asarray(x, dtype=np.float32)
    nc = _get_nc()
    in_maps = make_in_maps(x, wq, bq, wk, bk, wv, bv, wo, bo)
    res = run_bass_kernel_spmd(nc, in_maps, core_ids=list(range(NCORES)),
                               trace=_trace)
    if _results is not None:
        _results.append(res)
    # constant row: y += bv (since attn rows sum to 1)  =>  out += bv@wo + bo
    row_const = (np.asarray(bv, np.float64) @ np.asarray(wo, np.float64)
                 + np.asarray(bo, np.float64)).astype(np.float32)
    out = np.empty((B, T, D), dtype=np.float32)
    for b in range(B):
        out[b] = (res.results[2 * b]["out"] + res.results[2 * b + 1]["out"]
                  + row_const)
    return out
